# revision 13
# baseline (speedup 1.0000x reference)
"""GATNet (6 GAT layers + MLP head) on 8 Trainium2 NeuronCores — v3 (bf16).

Sharding: nodes/edges partitioned by destination across 8 cores. Host-side
load balancing permutes nodes into (core, block) bins with equalized edge
counts, so every 128-node block needs the same number of 128-edge chunks.

All features bf16. Per layer: transform h_ext = z @ [V|al_s|al_d] weights;
V|al_s go to h_own (AllGather'd in 5 groups of 4 blocks), al_d stays in
SBUF. Layer l+1's transform is fused into layer l's edge-phase epilogues,
consuming the freshly transposed z tiles straight from SBUF (no z
round-trip through DRAM), which also hides the grouped AllGathers under
the edge phase. Edge phase: dma_gather of src rows (dst-sorted,
chunk-aligned), per-chunk al_d via S^T matmul from the local block's al_d,
max-free segment softmax, segment-sum via 0/1 one-hot matmuls accumulating
in PSUM. Layer bias folded at the epilogue; fc1 fused into the epilogue.
Head: fc1 acc + BN fold, ReLU, one-hot pooling matmul, AllReduce, fc2,
lin, sigmoid.
"""
import os
import sys

sys.path.insert(0, "/opt/trn_rl_repo")

import numpy as np
import ml_dtypes
import concourse.bass as bass
import concourse.bacc as bacc
import concourse.mybir as mybir
import concourse.tile as tile
from concourse.bass_utils import run_bass_kernel_spmd

dt = mybir.dt
AF = mybir.ActivationFunctionType
ALU = mybir.AluOpType
BF16 = ml_dtypes.bfloat16

# ---------------------------------------------------------------- constants
N = 20000
E = 160000
G = 64
NCORES = 8
NPC = 2500                   # nodes per core (logical)
NPAD = 2560                  # padded (20 blocks of 128)
NBLK = NPAD // 128           # 20
LAYERS = [(3, 16, 8), (128, 16, 8), (128, 32, 8), (256, 32, 16), (512, 64, 16), (1024, 64, 16)]
HFS = [h * c for (_, c, h) in LAYERS]      # 128,128,256,512,1024,1024
HS = [h for (_, _, h) in LAYERS]
GWS = [256, 256, 384, 640, 1152, 1152]     # h_all row width (V|al_s|pad), mult of 128
TWS = [hf + 2 * h for hf, h in zip(HFS, HS)]  # 144,144,272,544,1056,1056
KINS = [4, 128, 128, 256, 512, 1024]       # L1 padded to K=4
ZOFFALL = [0, 128, 256, 512, 1024, 2048]   # fc1 row offset per layer
AGRP = 4                                   # blocks per AllGather group
ALL_ROWS = NCORES * NPAD                   # 20480
GCH = 8                                    # chunks per src-gather group


def _glob_row(core, loc):
    """h_all row of (core, local padded row): [grp][core][AGRP*128] layout."""
    b = loc // 128
    return (b // AGRP) * (NCORES * AGRP * 128) + core * (AGRP * 128) + (b % AGRP) * 128 + (loc % 128)


def _free_splits(w):
    out, o = [], 0
    while o < w:
        s = min(512, w - o)
        out.append((o, s))
        o += s
    return out


# ---------------------------------------------------------------- CPU prep
def balance_nodes(dst_full):
    """Assign nodes to (core, block) bins equalizing per-bin edge counts.

    dst_full includes self-loops. Returns (core, newloc) arrays [N]."""
    import heapq
    deg = np.bincount(dst_full, minlength=N).astype(np.int64)
    order = np.argsort(-deg, kind="stable")
    NBINS = NCORES * NBLK
    heap = [(0, b) for b in range(NBINS)]
    heapq.heapify(heap)
    cnt = np.zeros(NBINS, np.int64)
    binof = np.empty(N, np.int64)
    for n in order:
        while True:
            load, b = heapq.heappop(heap)
            if cnt[b] < 128:
                break
        binof[n] = b
        cnt[b] += 1
        heapq.heappush(heap, (load + int(deg[n]), b))
    order2 = np.argsort(binof, kind="stable")
    slot = np.empty(N, np.int64)
    slot[order2] = np.arange(N) - np.searchsorted(binof[order2], binof[order2])
    core = binof // NBLK
    blk = binof % NBLK
    newloc = blk * 128 + slot
    return core, newloc


def prep_edges(src, dst):
    """Per-core dst-sorted, block-aligned, core-uniform padded edge arrays."""
    s = np.concatenate([np.asarray(src, np.int64), np.arange(N, dtype=np.int64)])
    d = np.concatenate([np.asarray(dst, np.int64), np.arange(N, dtype=np.int64)])
    core, newloc = balance_nodes(d)
    s_row = _glob_row(core[s], newloc[s])           # h_all row of src
    d_core = core[d]
    d_loc = newloc[d]                               # local padded row of dst
    per_core = []
    cpb_all = np.zeros((NCORES, NBLK), np.int64)
    for r in range(NCORES):
        m = d_core == r
        es, ed = s_row[m], d_loc[m]
        o = np.argsort(ed, kind="stable")
        es, ed = es[o], ed[o]
        blk = ed // 128
        bl = [(es[blk == b], ed[blk == b]) for b in range(NBLK)]
        per_core.append(bl)
        cpb_all[r] = [(len(b[0]) + 127) // 128 for b in bl]
    cpb = cpb_all.max(axis=0)
    nch = int(cpb.sum())
    epad = nch * 128
    cores = []
    for r in range(NCORES):
        src_rows = np.zeros(epad, np.int64)
        dstloc = np.full(epad, -1.0, np.float32)
        o = 0
        for b in range(NBLK):
            bs, bd = per_core[r][b]
            k = len(bs)
            src_rows[o:o + k] = bs
            dstloc[o:o + k] = (bd - b * 128).astype(np.float32)
            o += int(cpb[b]) * 128
        cores.append((src_rows, dstloc))
    return cpb, nch, epad, cores, core, newloc


def _idx16(idx):
    a = np.asarray(idx).astype(np.int16).reshape(-1, 16).T
    return np.tile(a, (8, 1))               # [128, K/16]


def fold_weights(inp):
    w_ext, biases = [], []
    for i, (cin, cout, h) in enumerate(LAYERS):
        W = np.asarray(inp[f'W{i+1}'], np.float64)
        a_s = np.asarray(inp[f'as{i+1}'], np.float64)
        a_d = np.asarray(inp[f'ad{i+1}'], np.float64)
        hf = h * cout
        We = np.zeros((KINS[i], TWS[i]), np.float64)
        We[:cin, :hf] = W
        W3 = W.reshape(cin, h, cout)
        We[:cin, hf:hf + h] = np.einsum('chf,hf->ch', W3, a_s)
        We[:cin, hf + h:hf + 2 * h] = np.einsum('chf,hf->ch', W3, a_d)
        w_ext.append(We.astype(BF16))
        biases.append(np.asarray(inp[f'b{i+1}'], np.float32).reshape(1, hf).astype(BF16))
    sc = np.asarray(inp['bn_g'], np.float64) / np.sqrt(np.asarray(inp['bn_v'], np.float64) + 1e-5)
    fc1w = (np.asarray(inp['fc1_W'], np.float64) * sc[None, :]).astype(BF16)
    crow = ((np.asarray(inp['fc1_b'], np.float64) - np.asarray(inp['bn_m'], np.float64)) * sc
            + np.asarray(inp['bn_b'], np.float64)).astype(np.float32).reshape(1, 384)
    return w_ext, biases, fc1w, crow


# ---------------------------------------------------------------- program
def build_program(cpb, nch, epad):
    nc = bacc.Bacc("TRN2", target_bir_lowering=False, debug=False, num_devices=NCORES)

    # inputs
    xT0 = nc.dram_tensor("xT0", [4, NPAD], dt.bfloat16, kind="ExternalInput")
    w_in = [nc.dram_tensor(f"w{i+1}", [KINS[i], TWS[i]], dt.bfloat16, kind="ExternalInput")
            for i in range(6)]
    b_in = [nc.dram_tensor(f"b{i+1}", [1, HFS[i]], dt.bfloat16, kind="ExternalInput")
            for i in range(6)]
    fc1_in = nc.dram_tensor("fc1w", [3072, 384], dt.bfloat16, kind="ExternalInput")
    crow_in = nc.dram_tensor("crow", [1, 384], dt.float32, kind="ExternalInput")
    fc2_in = nc.dram_tensor("fc2w", [384, 256], dt.float32, kind="ExternalInput")
    fc2b_in = nc.dram_tensor("fc2b", [1, 256], dt.float32, kind="ExternalInput")
    lin_in = nc.dram_tensor("linw", [256, 1], dt.float32, kind="ExternalInput")
    linb_in = nc.dram_tensor("linb", [1, 1], dt.float32, kind="ExternalInput")
    gidx_in = nc.dram_tensor("gidx", [128, epad // 16], dt.int16, kind="ExternalInput")
    sall_in = nc.dram_tensor("sall", [128, nch * 128], dt.bfloat16, kind="ExternalInput")
    stra_in = nc.dram_tensor("stra", [128, nch * 128], dt.bfloat16, kind="ExternalInput")
    p1h_in = nc.dram_tensor("p1h", [NPAD, G], dt.bfloat16, kind="ExternalInput")
    cnti_in = nc.dram_tensor("cnti", [G, 1], dt.float32, kind="ExternalInput")
    identB_in = nc.dram_tensor("identB", [128, 128], dt.bfloat16, kind="ExternalInput")
    identF_in = nc.dram_tensor("identF", [128, 128], dt.float32, kind="ExternalInput")
    out_t = nc.dram_tensor("out", [G, 1], dt.float32, kind="ExternalOutput")

    chunk_blk = []
    for b in range(NBLK):
        chunk_blk += [b] * int(cpb[b])
    chunk_pos = []          # (is_first, is_last) within its block
    for b in range(NBLK):
        n = int(cpb[b])
        for k in range(n):
            chunk_pos.append((k == 0, k == n - 1))

    with tile.TileContext(nc) as tc:
        with tc.tile_pool(name="const", bufs=1) as cpool, \
             tc.tile_pool(name="wp", bufs=2) as wpool, \
             tc.tile_pool(name="hsb", bufs=2) as hsbp, \
             tc.tile_pool(name="gath", bufs=2) as gp, \
             tc.tile_pool(name="ework", bufs=2) as ep, \
             tc.tile_pool(name="sone", bufs=3) as sp, \
             tc.tile_pool(name="epi", bufs=2) as epip, \
             tc.tile_pool(name="bia", bufs=2) as bp, \
             tc.tile_pool(name="psbig", bufs=2, space="PSUM") as psb, \
             tc.tile_pool(name="psf1", bufs=2, space="PSUM") as f1p, \
             tc.tile_pool(name="dram", bufs=1, space="DRAM") as dram, \
             tc.tile_pool(name="dram2", bufs=2, space="DRAM") as dram2:

            # ---- constants
            identB = cpool.tile([128, 128], dt.bfloat16)
            nc.sync.dma_start(identB[:], identB_in[:])
            identF = cpool.tile([128, 128], dt.float32)
            nc.sync.dma_start(identF[:], identF_in[:])
            ones_sb = cpool.tile([1, 128], dt.float32)
            nc.vector.memset(ones_sb[:], 1.0)
            gidx_sb = cpool.tile([128, epad // 16], dt.int16)
            nc.sync.dma_start(gidx_sb[:], gidx_in[:])
            cnti_sb = cpool.tile([G, 1], dt.float32)
            nc.sync.dma_start(cnti_sb[:], cnti_in[:])
            xT0_sb = cpool.tile([4, NPAD], dt.bfloat16)
            nc.sync.dma_start(xT0_sb[:], xT0[:])
            p1h_sb = cpool.tile([128, NBLK * G], dt.bfloat16)
            nc.sync.dma_start(p1h_sb[:].rearrange("p (t g) -> p t g", g=G),
                              p1h_in[:].rearrange("(t p) g -> p t g", p=128))
            fc1w_sb = cpool.tile([128, 24 * 384], dt.bfloat16)
            nc.sync.dma_start(fc1w_sb[:].rearrange("p (k c) -> p k c", c=384),
                              fc1_in[:].rearrange("(k p) c -> p k c", p=128))
            crow_sb = cpool.tile([1, 384], dt.float32)
            nc.sync.dma_start(crow_sb[:], crow_in[:])
            c_tile = cpool.tile([128, 384], dt.float32)
            nc.gpsimd.partition_broadcast(c_tile[:], crow_sb[0:1, :])
            acc = cpool.tile([128, NBLK * 384], dt.float32)

            # per-layer state (filled by setup(li))
            st = {}

            def setup(li):
                """Load weights/bias, allocate h buffers for layer li."""
                HF, H, TW, KIN = HFS[li], HS[li], TWS[li], KINS[li]
                nk = max(1, KIN // 128)
                wt = wpool.tile([128, nk * TW], dt.bfloat16, tag="wt")
                if li == 0:
                    nc.sync.dma_start(wt[0:4, 0:TW], w_in[0][:, :])
                else:
                    for kb in range(nk):
                        nc.sync.dma_start(wt[:, kb * TW:(kb + 1) * TW],
                                          w_in[li][kb * 128:(kb + 1) * 128, :])
                brow = bp.tile([1, 1024], dt.bfloat16, tag="br")
                nc.sync.dma_start(brow[0:1, 0:HF], b_in[li][:, :])
                pb = bp.tile([128, 1024], dt.bfloat16, tag="pb")
                nc.gpsimd.partition_broadcast(pb[:, 0:HF], brow[0:1, 0:HF])
                ald_sb = bp.tile([128, NBLK * 16], dt.bfloat16, tag="alds")
                h_all = dram2.tile([ALL_ROWS, GWS[li]], dt.bfloat16, tag="hall")
                h_own = dram2.tile([NPAD, GWS[li]], dt.bfloat16, tag="hown")
                st[li] = dict(wt=wt, pb=pb, ald_sb=ald_sb, h_all=h_all, h_own=h_own)

            def transform_block(li, t, ts_prev):
                """Emit transform of layer li for block t.

                ts_prev: SBUF tile holding z_{li-1}^T k-blocks [128, nk*128]
                (None for li==0 → consumes xT0_sb)."""
                HF, H, TW, KIN = HFS[li], HS[li], TWS[li], KINS[li]
                nk = max(1, KIN // 128)
                S = st[li]
                ph = psb.tile([128, 1024], dt.float32, tag="big")
                aux = psb.tile([128, 32], dt.float32, tag="aux")
                if li == 0:
                    lhs0 = xT0_sb[:, t * 128:(t + 1) * 128]
                    nc.tensor.matmul(ph[:, 0:HF], lhs0, S["wt"][0:4, 0:HF],
                                     start=True, stop=True)
                    nc.tensor.matmul(aux[:, 0:2 * H], lhs0,
                                     S["wt"][0:4, HF:HF + 2 * H],
                                     start=True, stop=True)
                else:
                    wt = S["wt"]
                    for fo, fs in _free_splits(HF):
                        for kb in range(nk):
                            nc.tensor.matmul(
                                ph[:, fo:fo + fs], ts_prev[:, kb * 128:(kb + 1) * 128],
                                wt[:, kb * TW + fo:kb * TW + fo + fs],
                                start=(kb == 0), stop=(kb == nk - 1))
                    for kb in range(nk):
                        nc.tensor.matmul(
                            aux[:, 0:2 * H], ts_prev[:, kb * 128:(kb + 1) * 128],
                            wt[:, kb * TW + HF:kb * TW + HF + 2 * H],
                            start=(kb == 0), stop=(kb == nk - 1))
                hsag = hsbp.tile([128, GWS[li]], dt.bfloat16, tag="hsb")
                nc.scalar.copy(hsag[:, 0:HF], ph[:, 0:HF])
                nc.scalar.copy(hsag[:, HF:HF + H], aux[:, 0:H])
                nc.scalar.copy(S["ald_sb"][:, t * 16:t * 16 + H], aux[:, H:2 * H])
                nc.sync.dma_start(S["h_own"][t * 128:(t + 1) * 128, :], hsag[:])
                if t % AGRP == AGRP - 1:
                    g = t // AGRP
                    nc.gpsimd.collective_compute(
                        "AllGather", ALU.bypass,
                        replica_groups=[list(range(NCORES))],
                        ins=[S["h_own"][g * AGRP * 128:(g + 1) * AGRP * 128, :].opt()],
                        outs=[S["h_all"][g * NCORES * AGRP * 128:
                                         (g + 1) * NCORES * AGRP * 128, :].opt()])

            def edge_phase(li):
                """Edge phase of layer li; fuses transform of li+1 into the
                epilogues (and the grouped AllGathers of li+1 behind it)."""
                HF, H, GW = HFS[li], HS[li], GWS[li]
                F = HF // H
                nfb = HF // 128
                S = st[li]
                h_all, ald_sb, pb = S["h_all"], S["ald_sb"], S["pb"]
                apsum = dps = None

                def prologue(g0):
                    """Gather + attention scores + V-scaling for one group."""
                    gc = min(GCH, nch - g0)
                    gt3 = gp.tile([128, GCH, GW], dt.bfloat16, tag="gt")
                    nc.gpsimd.dma_gather(
                        gt3[:, 0:gc, :], h_all[:, :], gidx_sb[:, g0 * 8:(g0 + gc) * 8],
                        gc * 128, gc * 128, elem_size=GW, single_packet=False)
                    # per-chunk al_d via S^T matmul from the local block row
                    stg = sp.tile([128, GCH * 128], dt.bfloat16, tag="St")
                    nc.sync.dma_start(stg[:, 0:gc * 128],
                                      stra_in[:, g0 * 128:(g0 + gc) * 128])
                    edg = ep.tile([128, GCH, 16], dt.bfloat16, tag="edg")
                    for lc in range(gc):
                        blk = chunk_blk[g0 + lc]
                        edp = f1p.tile([128, 16], dt.float32, tag="f1")
                        nc.tensor.matmul(edp[:, 0:H],
                                         stg[:, lc * 128:(lc + 1) * 128],
                                         ald_sb[:, blk * 16:blk * 16 + H],
                                         start=True, stop=True)
                        nc.scalar.copy(edg[:, lc, 0:H], edp[:, 0:H])
                    # e = al_s + al_d ; exp(lrelu(e)) into al_s cols of gt
                    et = ep.tile([128, GCH, 16], dt.bfloat16, tag="et")
                    e3 = et[:, 0:gc, 0:H]
                    nc.vector.tensor_tensor(e3, gt3[:, 0:gc, HF:HF + H],
                                            edg[:, 0:gc, 0:H], op=ALU.add)
                    xs = ep.tile([128, GCH, 16], dt.bfloat16, tag="xs")
                    x3 = xs[:, 0:gc, 0:H]
                    nc.scalar.activation(x3, e3, AF.Lrelu, alpha=0.2)
                    nc.scalar.activation(gt3[:, 0:gc, HF:HF + H], x3, AF.Exp)
                    sgt = sp.tile([128, GCH * 128], dt.bfloat16, tag="S")
                    nc.sync.dma_start(sgt[:, 0:gc * 128],
                                      sall_in[:, g0 * 128:(g0 + gc) * 128])
                    for lc in range(gc):
                        # weighted V for each chunk
                        v3 = gt3[:, lc, 0:HF].rearrange("p (h f) -> p h f", h=H)
                        ex3 = gt3[:, lc, HF:HF + H].broadcast_to((128, H, F))
                        nc.vector.tensor_tensor(v3, v3, ex3, op=ALU.mult)
                    return (g0, gc, gt3, sgt)

                def scatter(state):
                    nonlocal apsum, dps
                    g0, gc, gt3, sgt = state
                    for c in range(g0, g0 + gc):
                        first, last = chunk_pos[c]
                        blk = chunk_blk[c]
                        if first:
                            apsum = psb.tile([128, 1024], dt.float32, tag="big")
                            dps = psb.tile([128, 16], dt.float32, tag="aux")
                        lc = c - g0
                        for fo, fs in _free_splits(HF):
                            nc.tensor.matmul(apsum[:, fo:fo + fs],
                                             sgt[:, lc * 128:(lc + 1) * 128],
                                             gt3[:, lc, fo:fo + fs],
                                             start=first, stop=last)
                        nc.tensor.matmul(dps[:, 0:H],
                                         sgt[:, lc * 128:(lc + 1) * 128],
                                         gt3[:, lc, HF:HF + H],
                                         start=first, stop=last)
                        if last:
                            # epilogue: divide by denom, add bias, transpose,
                            # fused fc1, then fused transform of layer li+1
                            rt = epip.tile([128, 16], dt.float32, tag="rt")
                            nc.vector.tensor_scalar(rt[:, 0:H], dps[:, 0:H],
                                                    1e-16, None, op0=ALU.add)
                            rec = epip.tile([128, 16], dt.float32, tag="rec")
                            nc.vector.reciprocal(rec[:, 0:H], rt[:, 0:H])
                            osb = epip.tile([128, 1024], dt.bfloat16, tag="osb")
                            o4 = osb[:, 0:HF].rearrange("p (h f) -> p h f", h=H)
                            p4 = apsum[:, 0:HF].rearrange("p (h f) -> p h f", h=H)
                            r4 = rec[:, 0:H].broadcast_to((128, H, F))
                            nc.vector.tensor_tensor(o4, p4, r4, op=ALU.mult)
                            nc.vector.tensor_tensor(osb[:, 0:HF], osb[:, 0:HF],
                                                    pb[:, 0:HF], op=ALU.add)
                            ts = epip.tile([128, 8 * 128], dt.bfloat16, tag="ts")
                            for fb in range(nfb):
                                tp = psb.tile([128, 128], dt.bfloat16, tag="aux")
                                nc.tensor.transpose(
                                    tp[:], osb[:, fb * 128:(fb + 1) * 128], identB[:])
                                nc.scalar.copy(ts[:, fb * 128:(fb + 1) * 128], tp[:])
                            kb0 = ZOFFALL[li] // 128
                            f1 = f1p.tile([128, 384], dt.float32, tag="f1")
                            for fb in range(nfb):
                                nc.tensor.matmul(
                                    f1[:, 0:384], ts[:, fb * 128:(fb + 1) * 128],
                                    fc1w_sb[:, (kb0 + fb) * 384:(kb0 + fb + 1) * 384],
                                    start=(fb == 0), stop=(fb == nfb - 1))
                            asl = acc[:, blk * 384:(blk + 1) * 384]
                            if li == 0:
                                nc.scalar.copy(asl, f1[:, 0:384])
                            else:
                                nc.vector.tensor_tensor(asl, asl, f1[:, 0:384],
                                                        op=ALU.add)
                            if li < 5:
                                transform_block(li + 1, blk, ts)

                # software pipeline: prologue of group g+1 ahead of scatter(g)
                prev = None
                for g0 in range(0, nch, GCH):
                    cur = prologue(g0)
                    if prev is not None:
                        scatter(prev)
                    prev = cur
                scatter(prev)

            # ---------------- main schedule ----------------
            setup(0)
            for t in range(NBLK):
                transform_block(0, t, None)
            for li in range(6):
                if li < 5:
                    setup(li + 1)
                edge_phase(li)

            # ================= head =================
            poolp = f1p.tile([G, 384], dt.float32, tag="f1")
            for t in range(NBLK):
                tmp = epip.tile([128, 384], dt.float32, tag="tmp")
                nc.vector.tensor_tensor(tmp[:], acc[:, t * 384:(t + 1) * 384],
                                        c_tile[:], op=ALU.add)
                ra = epip.tile([128, 384], dt.bfloat16, tag="ra")
                nc.scalar.activation(ra[:], tmp[:], AF.Relu)
                nc.tensor.matmul(poolp[:], p1h_sb[:, t * G:(t + 1) * G], ra[:],
                                 start=(t == 0), stop=(t == NBLK - 1))

            pool_sb = cpool.tile([G, 384], dt.float32)
            nc.scalar.copy(pool_sb[:], poolp[:])
            ar_in = dram.tile([G, 384], dt.float32)
            ar_out = dram.tile([G, 384], dt.float32, addr_space="Shared")
            nc.gpsimd.dma_start(ar_in[:], pool_sb[:])
            nc.gpsimd.collective_compute(
                "AllReduce", ALU.add, replica_groups=[list(range(NCORES))],
                ins=[ar_in.opt()], outs=[ar_out.opt()])
            pool2 = cpool.tile([G, 384], dt.float32)
            nc.gpsimd.dma_start(pool2[:], ar_out[:])
            pool3 = cpool.tile([G, 384], dt.float32)
            nc.vector.tensor_scalar(pool3[:], pool2[:], cnti_sb[:, 0:1], None,
                                    op0=ALU.mult)

            # transpose pooled -> [384, 64]
            pTs = cpool.tile([128, 3, G], dt.float32)
            for fb in range(3):
                tp = f1p.tile([128, 128], dt.float32, tag="f1")
                nc.tensor.transpose(tp[0:128, 0:G], pool3[:, fb * 128:(fb + 1) * 128],
                                    identF[0:G, 0:G])
                nc.scalar.copy(pTs[:, fb, :], tp[0:128, 0:G])

            fc2_sb = cpool.tile([128, 3 * 256], dt.float32)
            nc.sync.dma_start(fc2_sb[:].rearrange("p (k c) -> p k c", c=256),
                              fc2_in[:].rearrange("(k p) c -> p k c", p=128))
            fc2b_sb = cpool.tile([1, 256], dt.float32)
            nc.sync.dma_start(fc2b_sb[:], fc2b_in[:])
            lin_sb = cpool.tile([128, 2], dt.float32)
            nc.sync.dma_start(lin_sb[:].rearrange("p (k c) -> p k c", c=1),
                              lin_in[:].rearrange("(k p) c -> p k c", p=128))
            linb_sb = cpool.tile([1, 1], dt.float32)
            nc.sync.dma_start(linb_sb[:], linb_in[:])

            p2 = f1p.tile([G, 256], dt.float32, tag="f1")
            for kb in range(3):
                nc.tensor.matmul(p2[:], pTs[:, kb, :], fc2_sb[:, kb * 256:(kb + 1) * 256],
                                 start=(kb == 0), stop=False)
            nc.tensor.matmul(p2[:], ones_sb[0:1, 0:G], fc2b_sb[:], start=False, stop=True)
            r2 = cpool.tile([G, 256], dt.float32)
            nc.scalar.activation(r2[:], p2[:], AF.Relu)

            rTs = cpool.tile([128, 2, G], dt.float32)
            for fb in range(2):
                tp = f1p.tile([128, 128], dt.float32, tag="f1")
                nc.tensor.transpose(tp[0:128, 0:G], r2[:, fb * 128:(fb + 1) * 128],
                                    identF[0:G, 0:G])
                nc.scalar.copy(rTs[:, fb, :], tp[0:128, 0:G])

            p3 = f1p.tile([G, 1], dt.float32, tag="f1")
            for kb in range(2):
                nc.tensor.matmul(p3[:], rTs[:, kb, :], lin_sb[:, kb:kb + 1],
                                 start=(kb == 0), stop=False)
            nc.tensor.matmul(p3[:], ones_sb[0:1, 0:G], linb_sb[:], start=False, stop=True)
            res = cpool.tile([G, 1], dt.float32)
            nc.scalar.activation(res[:], p3[:], AF.Sigmoid)
            nc.sync.dma_start(out_t[:], res[:])

    nc.compile()
    return nc


# ---------------------------------------------------------------- driver
_CACHE = {}


def kernel(**inputs):
    trace = bool(inputs.pop("_trace", False))
    inp = {k: np.asarray(v) for k, v in inputs.items() if k != "num_graphs"}
    src, dst = inp['src'], inp['dst']
    batch = np.asarray(inp['batch']).astype(np.int64)
    x = np.asarray(inp['x'], np.float32)

    cpb, nch, epad, cores, core_of, newloc = prep_edges(src, dst)
    w_ext, biases, fc1w, crow = fold_weights(inp)

    key = (tuple(cpb),)
    if key not in _CACHE:
        _CACHE[key] = build_program(cpb, nch, epad)
    nc = _CACHE[key]

    cnt = np.bincount(batch, minlength=G).astype(np.float64)
    cnti = (1.0 / np.maximum(cnt, 1.0)).astype(np.float32).reshape(G, 1)
    identB = np.eye(128, dtype=np.float32).astype(BF16)
    identF = np.eye(128, dtype=np.float32)

    in_maps = []
    for r in range(NCORES):
        m = core_of == r
        xa = np.zeros((4, NPAD), np.float32)
        xa[0:3, newloc[m]] = x[m].T
        src_rows, dstloc = cores[r]
        p1h = np.zeros((NPAD, G), np.float32)
        p1h[newloc[m], batch[m]] = 1.0
        onehot = (dstloc.reshape(nch, 128)[:, :, None] ==
                  np.arange(128, dtype=np.float32)[None, None, :]).astype(BF16)
        mm = {
            "xT0": xa.astype(BF16),
            "fc1w": fc1w,
            "crow": crow,
            "fc2w": np.asarray(inp['fc2_W'], np.float32),
            "fc2b": np.asarray(inp['fc2_b'], np.float32).reshape(1, 256),
            "linw": np.asarray(inp['lin_W'], np.float32),
            "linb": np.asarray(inp['lin_b'], np.float32).reshape(1, 1),
            "gidx": _idx16(src_rows),
            "sall": onehot.transpose(1, 0, 2).reshape(128, nch * 128),
            "stra": onehot.transpose(2, 0, 1).reshape(128, nch * 128),
            "p1h": p1h.astype(BF16),
            "cnti": cnti,
            "identB": identB,
            "identF": identF,
        }
        for i in range(6):
            mm[f"w{i+1}"] = w_ext[i]
            mm[f"b{i+1}"] = biases[i]
        in_maps.append(mm)

    res = run_bass_kernel_spmd(nc, in_maps, list(range(NCORES)), trace=trace)
    out = res.results[0]["out"].reshape(G, 1).astype(np.float32)
    if trace:
        return out, res
    return out
